# revision 12
# baseline (speedup 1.0000x reference)
"""Chunked attention Trainium2 Bass kernel.

Problem: B=2, S=8192, HIDDEN=1024, HEADS=16, HEAD_DIM=64, CHUNK=2048,
OVERLAP=128. Sharding: head-parallel x batch-parallel -> 32 (b,h) jobs,
4 per core on 8 cores. Each core computes full-seq chunked attention for
its 4 heads; the host slices/pre-transposes inputs and reassembles the
output.

Per-core dataflow:
  - Host supplies Q^T and K^T in [d, seq] bf16 layout, duplicated across
    both 64-partition halves so K_c=64 matmuls can be row-packed in pairs
    (two concurrent matmuls in the PE array). Q is pre-scaled by
    SCALE*H_EXP on the host.
  - QK^T: S^T[k,q] bf16 matmuls into PSUM groups of 3 banks (bf16 also
    enables FWL so the 128-col weight loads hide under the matmuls).
  - Softmax exp is split across TWO engines (the scalar/ACT engine is
    otherwise the bottleneck): per k-group, an 8:7 weighted round-robin
    assigns either
      ACT:  exp(w * 1/H_EXP)  (table-based, exact), or
      DVE:  custom op EXP_SQ8_ANT = (((w+A)w+B)w+C)^8 — a monic cubic
            with implied scale 1 followed by three squarings; max rel
            err ~1.7e-2 on the DVE share only.
  - PV: lhsT=[V|1] (65 cols, fp32r stationary) accumulates [O^T; l] into
    one PSUM bank over all k-tiles of the chunk; pT is consumed as f32r.
  - PSUM->SBUF output staging runs on the (otherwise idle) Pool engine.
  - Device returns the UNNORMALIZED per-chunk [O^T; l] (65 rows per
    chunk, concatenated along seq); softmax division and the 128-wide
    overlap-band blending happen on the host in fp32.
"""

import sys

if '/opt/trn_rl_repo' not in sys.path:
    sys.path.insert(0, '/opt/trn_rl_repo')

import numpy as np

import concourse.bass as bass
import concourse.mybir as mybir
import concourse.tile as tile
from concourse.bass_utils import run_bass_kernel_spmd

F32 = mybir.dt.float32
F32R = mybir.dt.float32r
BF16 = mybir.dt.bfloat16
EXP = mybir.ActivationFunctionType.Exp

B, S, HIDDEN, HEADS, HD = 2, 8192, 1024, 16, 64
SCALE = 1.0 / 8.0  # 1/sqrt(64)
N_CORES = 8
JOBS = 4  # (b, h) pairs per core
# (q0, Lq, k0, Lk) per chunk; step=1920, overlap=128
CHUNKS = [
    (0, 2048, 0, 2176),
    (1920, 2048, 1792, 2304),
    (3840, 2048, 3712, 2304),
    (5760, 2048, 5632, 2304),
    (7680, 512, 7552, 640),
]
COLS = [0, 2048, 4096, 6144, 8192]  # chunk col offsets in the out buffer
SQ = 8704  # sum of chunk Lq
GROUP = int(__import__('os').environ.get('QK_GROUP', '3'))
SP_BUFS = int(__import__('os').environ.get('SP_BUFS', '2'))  # GROUP*SP_BUFS + 2 <= 8

# ---- exp split constants -------------------------------------------------
# DVE path: w = s*H_EXP; p8 = (((w+CA)w+CB)w+CC)^8 ~ e^s (monic cubic with
# implied scale 1; fitted on s in [-6.2, 6.2], gauss-weighted minimax).
H_EXP = float((1.0 / 6.0) ** (1.0 / 3.0) / 8.0)
ACT_SCALE = float(1.0 / H_EXP)
CA = 1.7404664027630232
CB = 1.8227268367485119
CC = 0.9980881316679443
HOST_Q_SCALE = SCALE * H_EXP  # folded into qT on the host
# exp engine split: per Q-BLOCK (not per group) so each PV accumulation
# chain consumes pT from a single engine (mixed producers forced the PE
# to carry >1 sync-waits -> NoOp stalls). The custom DVE op measures
# ~2 elem/cycle/lane, so DVE gets the bigger share: ACT:DVE = 3:5.
import os as _os
ACT_OF_8 = int(_os.environ.get('ACT_OF_8', '3'))
QK_BF16 = _os.environ.get('QK_BF16', '0') == '1'
QK_DT = BF16 if QK_BF16 else F32


def _use_act(qi: int) -> bool:
    return (qi * ACT_OF_8) % 8 < ACT_OF_8


# ---- custom DVE op -------------------------------------------------------

def _make_exp_op():
    """Register EXP_SQ8_ANT in dve_ops.OPS (idempotent, process-local)."""
    import concourse.dve_ops as dve_ops
    from concourse.dve_ops import DveOp
    from concourse.dve_spec import C0, C1, C2, Spec, Src0, lower, sq
    from concourse.dve_uop import DveOpSpec

    name = "EXP_SQ8_ANT"
    for op in dve_ops.OPS:
        if op.name == name:
            return op

    def _ref(in0, in1, c0, c1, c2):
        w = in0.astype(np.float32)
        p = (((w + np.float32(c0)) * w + np.float32(c1)) * w
             + np.float32(c2)).astype(np.float32)
        for _ in range(3):
            p = (p * p).astype(np.float32)
        return p

    spec = Spec(body=sq(sq(sq(((Src0 + C0) * Src0 + C1) * Src0 + C2))),
                reference=_ref)
    row = dve_ops._CUSTOM_DVE_ROW_BASE + len(dve_ops.OPS)
    assert row < 0x20
    dve_ops._SUB_OPCODE_FOR_NAME[name] = row
    shas = {}
    for ver in ("v3", "v4"):
        try:
            s = DveOpSpec(name=name, opcode=row, uops=lower(spec, ver=ver),
                          rd1_en=False)
            shas[ver] = s.sha(ver)
        except Exception:
            if ver == "v3":
                raise
    op = DveOp(name, spec, subdim=False, uops_sha=shas)
    dve_ops.OPS.append(op)
    dve_ops.CUSTOM_DVE_SPECS[name] = spec
    return op


def _legalize_waits(nc, max_waits=1):
    """walrus in this config rejects >1 sync-wait per instruction: hoist
    excess waits onto injected same-engine NoOps placed just before."""
    cnt = 0
    for f in nc.m.functions:
        for blk in f.blocks:
            il = blk.instructions
            if not any(
                i.sync_info is not None and i.sync_info.on_wait
                and len(i.sync_info.on_wait) > max_waits for i in il
            ):
                continue
            new = []
            for inst in il:
                si = inst.sync_info
                if si is not None and si.on_wait and len(si.on_wait) > max_waits:
                    waits = list(si.on_wait)
                    spill, keep = waits[:-max_waits], waits[-max_waits:]
                    for w in spill:
                        nop = mybir.InstNoOp(
                            name=f"I-wsplit-{cnt}", ins=[], outs=[])
                        cnt += 1
                        nop.engine = inst.engine
                        nop.sync_info = mybir.SyncInfo(on_wait=[w], on_update=[])
                        new.append(nop)
                    inst.sync_info = mybir.SyncInfo(
                        on_wait=keep, on_update=list(si.on_update or []))
                new.append(inst)
            blk.instructions = new
    return cnt


def _build_nc(reps=1):
    exp_op = _make_exp_op()
    nc = bass.Bass()
    qt_in = nc.declare_dram_parameter("qt", [JOBS, 128, S], QK_DT, isOutput=False)
    kt_in = nc.declare_dram_parameter("kt", [JOBS, 128, S], QK_DT, isOutput=False)
    v_in = nc.declare_dram_parameter("v", [JOBS, S, HD], F32, isOutput=False)
    out = nc.declare_dram_parameter("out", [JOBS, 65, SQ], F32, isOutput=True)

    with tile.TileContext(nc) as tc:
        with (
            tc.tile_pool(name="const", bufs=1) as cpool,
            tc.tile_pool(name="ops", bufs=2) as ops,          # qT/kT/vW
            tc.tile_pool(name="probs", bufs=7) as probs,      # pT
            tc.tile_pool(name="opath", bufs=3) as opath,      # o_sb staging
            tc.tile_pool(name="spsum", bufs=SP_BUFS, space="PSUM") as spsum,
            tc.tile_pool(name="onepsum", bufs=2, space="PSUM") as onepsum,
        ):
            ones_f32 = cpool.tile([128, 1], F32)
            nc.vector.memset(ones_f32, 1.0)

            qi = 0  # global q-block counter for the ACT/DVE pattern
            for j in [jj for _ in range(reps) for jj in range(JOBS)]:
                for ci, (q0, lq, k0, lk) in enumerate(CHUNKS):
                    nk = lk // 128

                    if QK_BF16:
                        qT = ops.tile([128, lq], BF16, tag="qT")
                        nc.sync.dma_start(out=qT, in_=qt_in[j, :, q0:q0 + lq])
                        kT = ops.tile([128, lk], BF16, tag="kT")
                        nc.sync.dma_start(out=kT, in_=kt_in[j, :, k0:k0 + lk])
                    else:
                        qT = ops.tile([128, lq], F32R, tag="qT")
                        nc.sync.dma_start(
                            out=qT, in_=qt_in[j, :, q0:q0 + lq].bitcast(F32R))
                        kT = ops.tile([128, lk], F32R, tag="kT")
                        nc.sync.dma_start(
                            out=kT, in_=kt_in[j, :, k0:k0 + lk].bitcast(F32R))
                    vW = ops.tile([128, nk * 65], F32R, tag="vW")
                    vv = vW.rearrange("p (t e) -> p t e", e=65)
                    nc.sync.dma_start(
                        out=vv[:, :, 0:64],
                        in_=v_in[j, k0:k0 + lk, :].rearrange(
                            "(t p) d -> p t d", p=128).bitcast(F32R),
                    )
                    nc.vector.tensor_copy(
                        vv[:, :, 64], ones_f32.broadcast_to([128, nk]))

                    ngroups = (nk + GROUP - 1) // GROUP
                    for qb in range(lq // 512):
                        qs = slice(qb * 512, qb * 512 + 512)
                        use_act = _use_act(qi)
                        qi += 1
                        opsum = onepsum.tile([128, 512], F32, tag="opsum")
                        # emit all QK+exp for the q-block first so the PE
                        # always feeds the exp engines before doing PV work
                        pTs = []
                        for g in range(ngroups):
                            kts = list(range(g * GROUP, min((g + 1) * GROUP, nk)))
                            sp = spsum.tile([128, 512 * GROUP], F32, tag="sp")
                            # QK^T: S^T[k,q]; consecutive k-tiles alternate
                            # row halves -> pairs run concurrently in PE
                            for i, kt in enumerate(kts):
                                rows = slice(64 * (kt % 2), 64 * (kt % 2) + 64)
                                nc.tensor.matmul(
                                    sp[:, i * 512:(i + 1) * 512],
                                    kT[rows, kt * 128:(kt + 1) * 128],
                                    qT[rows, qs],
                                    start=True, stop=True,
                                    tile_position=(64 * (kt % 2), 0),
                                    skip_group_check=True,
                                )
                            pT = probs.tile([128, 512 * GROUP], F32R, tag="pT")
                            nw = 512 * len(kts)
                            if use_act:
                                nc.scalar.activation(
                                    pT[:, 0:nw], sp[:, 0:nw], EXP,
                                    scale=ACT_SCALE)
                            else:
                                nc.vector._custom_dve(
                                    exp_op, out=pT[:, 0:nw], in0=sp[:, 0:nw],
                                    s0=CA, s1=CB, imm2=CC)
                            pTs.append((kts, pT))
                        for kts, pT in pTs:
                            for i, kt in enumerate(kts):
                                nc.tensor.matmul(
                                    opsum[0:65, :],
                                    vW[:, kt * 65:(kt + 1) * 65],
                                    pT[:, i * 512:(i + 1) * 512],
                                    start=(kt == 0), stop=(kt == nk - 1),
                                    skip_group_check=True,
                                )
                        o_sb = opath.tile([65, 512], F32, tag="osb")
                        nc.scalar.copy(o_sb, opsum[0:65, :])
                        c0 = COLS[ci] + qb * 512
                        nc.sync.dma_start(
                            out=out[j, :, c0:c0 + 512], in_=o_sb)

    mybir.codegen_inst_isa_subclasses(nc)
    _legalize_waits(nc)
    return nc


_NC = None


def _get_nc():
    global _NC
    if _NC is None:
        _NC = _build_nc()
    return _NC


def make_in_maps(query, key_, value):
    """Host-side prep: per-core slices; Q^T/K^T in [d, seq] bf16 layout
    duplicated across both partition halves. Q pre-scaled by SCALE*H_EXP
    (the exp-input normalization for both engines)."""
    bf16 = mybir.dt.np(BF16) if QK_BF16 else np.float32
    qh = query.reshape(B, S, HEADS, HD)
    kh = key_.reshape(B, S, HEADS, HD)
    vh = value.reshape(B, S, HEADS, HD)
    qT = np.ascontiguousarray(
        qh.transpose(0, 2, 3, 1)) * np.float32(HOST_Q_SCALE)  # [B, H, D, S]
    qT = qT.astype(bf16)
    kT = np.ascontiguousarray(kh.transpose(0, 2, 3, 1)).astype(bf16)
    in_maps = []
    for c in range(N_CORES):
        jobs = [(g // HEADS, g % HEADS) for g in range(4 * c, 4 * c + 4)]
        qt_c = np.empty((JOBS, 128, S), bf16)
        kt_c = np.empty((JOBS, 128, S), bf16)
        v_c = np.empty((JOBS, S, HD), np.float32)
        for jj, (b, h) in enumerate(jobs):
            qt_c[jj, 0:64] = qT[b, h]
            qt_c[jj, 64:128] = qT[b, h]
            kt_c[jj, 0:64] = kT[b, h]
            kt_c[jj, 64:128] = kT[b, h]
            v_c[jj] = vh[b, :, h]
        in_maps.append({"qt": qt_c, "kt": kt_c, "v": v_c})
    return in_maps


def assemble_out(results):
    """Host: per-chunk softmax division + overlap-band blending (fp32,
    mirrors the reference's merge), then scatter into [B, S, HIDDEN]."""
    wt = np.linspace(1.0, 0.0, 128).astype(np.float32)  # prev-chunk tail
    wh = np.linspace(0.0, 1.0, 128).astype(np.float32)  # cur-chunk head
    denom = (wt + wh) + np.float32(1e-10)
    a = (wt / denom).astype(np.float32)[:, None]
    bb = (wh / denom).astype(np.float32)[:, None]

    out = np.empty((B, S, HIDDEN), dtype=np.float32)
    for c in range(N_CORES):
        oc = results[c]["out"]  # [4, 65, SQ]
        for jj, g in enumerate(range(4 * c, 4 * c + 4)):
            b, h = g // HEADS, g % HEADS
            full = np.empty((S, HD), np.float32)
            prev_tail = None
            for ci, (q0, lq, k0, lk) in enumerate(CHUNKS):
                off = COLS[ci]
                blk = oc[jj, :, off:off + lq]
                on = (blk[0:64] / blk[64:65]).T  # [lq, 64] normalized
                lo = 0
                if ci > 0:
                    full[q0:q0 + 128] = prev_tail * a + on[0:128] * bb
                    lo = 128
                hi = lq
                if ci < len(CHUNKS) - 1:
                    hi = lq - 128
                    prev_tail = on[lq - 128:lq]
                full[q0 + lo:q0 + hi] = on[lo:hi]
            out[b, :, h * HD:(h + 1) * HD] = full
    return out


def kernel(query, key, value):
    query = np.asarray(query, dtype=np.float32)
    key_ = np.asarray(key, dtype=np.float32)
    value = np.asarray(value, dtype=np.float32)
    nc = _get_nc()
    in_maps = make_in_maps(query, key_, value)
    res = run_bass_kernel_spmd(nc, in_maps, list(range(N_CORES)))
    return assemble_out(res.results)
